# revision 18
# baseline (speedup 1.0000x reference)
"""Causal multi-head self-attention on 8 TRN2 NeuronCores (Bass/Tile).

Problem: x[2,2048,1024] -> Attention(16 heads x 64) with causal mask -> out[2,2048,1024].

Sharding (batch x head parallel): core c owns batch c//4 and heads
[4*(c%4), 4*(c%4)+4) — 4 heads = 256 inner features, processed as TWO
head-pairs (hp in {0,1}, 128 features each, the same per-pair shape the
2-heads-per-core layout used).  Halves both the input DMA (one batch's
x = 4 MB instead of 8) and the output DMA (one batch's partial = 4 MB):
the DMA engines were saturated for the first ~22us under the old
full-x-per-core layout.
  - Wq/Wk/Wv column slices [1024, 256], Wo row slice [256, 1024]
  - each core computes a partial output [2048, 1024] for its batch; the
    host sums the 4 partials per batch and adds the output bias.

Device algorithm per core (all attention matmuls bf16):
  - host pre-arranges the core's batch of x into tile-contiguous
    xt [2, 8, 128, 1024] (bf16, dim-on-partitions).
  - PE warm-up dummy matmuls at t=0 (HAM clock gate, see baseline notes).
  - per head-pair hp: qT, kT [128(2 heads*64), 2048] = Wslice.T @ x.T
  - V^T computed then PE-transposed into v tiles [128, 130] = [V_h0|1|V_h1|1].
  - S^T tiles [j=128, i=512] per head; P^T = exp(S^T * scale) on ACT;
    exact-causal via band width + gpsimd affine_select on diagonal tiles.
  - O^T accumulation via [V|1] matmuls -> rows 0:64 = O^T, row 64 = denom r.
  - normalization on PSUM evacuation: rank-1 bf16 matmul broadcast of r,
    reciprocal, tensor_mul (bf16 stationary: fp32r rank-1 matmuls measured
    ~2x slower on HW).
  - out-proj accumulates BOTH head-pairs into one PSUM tile per
    (token-tile, 512-col chunk): partial[tok,1024] = sum_hp oT_hp.T @ Wo_hp.
  - block order (hp,bi): (0,0),(0,1),(0,2),(0,3),(1,1),(1,2),(1,3),(1,0) —
    the kernel drains on a 4-j-tile block instead of a 16-j-tile one, so the
    final exp chain + evacuation + out-proj tail is ~3x shorter.

Scheduling: identical machinery to the tuned baseline (0.9us filler thunks
after each j-tile's PV, DMA-issue spreading across sync/scalar/gpsimd in
the preamble, out-DMAs on sync only, keep-warm dummies).  PSUM: stp
2x[128,1024] + pprj 2x[128,512] + pacc 2x[128,512] = 8 banks.
"""

import numpy as np

import concourse.bass as bass
import concourse.mybir as mybir
from concourse import bacc
import concourse.tile as tile
from concourse.masks import make_identity

F32 = mybir.dt.float32
F32R = mybir.dt.float32r
BF16 = mybir.dt.bfloat16
EXP = mybir.ActivationFunctionType.Exp

# problem constants
B = 2
N = 2048
DIM = 1024
HEADS = 16
DH = 64
INNER = HEADS * DH
SCALE = DH ** -0.5
NCORES = 8
HPB = 4                    # head-quads: cores per batch
HPC = HEADS // NCORES      # heads per pair = 2
FPP = HPC * DH             # features per pair = 128
NPAIR = 2                  # head-pairs per core

TRACE = False
LAST_EXEC_NS = None

_nc_cache = {}


def build_nc(n=N, dim=DIM):
    """Build the per-core Bass program (identical on all 8 cores)."""
    kc_n = dim // 128          # contraction chunks (8)
    ntb = n // 512             # 512-wide token blocks (4)
    nbi = n // 512             # attention i-blocks (4)
    nxh = n // 1024            # 1024-wide xt half-blocks (2)

    nc = bacc.Bacc(None)
    # tile-contiguous input (one batch): [half, kc, 128, 1024]
    xt_d = nc.dram_tensor("xt", [nxh, kc_n, 128, 1024], BF16, kind="ExternalInput")
    # pair-major weights: [128, pair, kc, 128] so each head-pair's slice is
    # one contiguous 256 KB DMA (the startup chain only needs pair 0's q/k)
    wq = nc.dram_tensor("wq", [128, NPAIR, kc_n, FPP], BF16, kind="ExternalInput")
    wk = nc.dram_tensor("wk", [128, NPAIR, kc_n, FPP], BF16, kind="ExternalInput")
    wv = nc.dram_tensor("wv", [128, NPAIR, kc_n, FPP], BF16, kind="ExternalInput")
    wo = nc.dram_tensor("wo", [128, NPAIR, dim], BF16, kind="ExternalInput")
    out = nc.dram_tensor("out", [n, dim], BF16, kind="ExternalOutput")

    with tile.TileContext(nc) as tc, \
         tc.tile_pool(name="singles", bufs=1) as singles, \
         tc.tile_pool(name="xtp", bufs=nxh * kc_n) as xtp, \
         tc.tile_pool(name="qkp", bufs=NPAIR * ntb * 2) as qkp, \
         tc.tile_pool(name="vsp", bufs=2) as vsp, \
         tc.tile_pool(name="vp", bufs=NPAIR * 4 * ntb) as vp, \
         tc.tile_pool(name="ptp", bufs=6) as ptp, \
         tc.tile_pool(name="rp", bufs=4) as rp, \
         tc.tile_pool(name="ostp", bufs=6) as ostp, \
         tc.tile_pool(name="otp", bufs=NPAIR * nbi) as otp, \
         tc.tile_pool(name="osh", bufs=2) as osh, \
         tc.tile_pool(name="pstp", bufs=2, space="PSUM") as pstp, \
         tc.tile_pool(name="pprj", bufs=2, space="PSUM") as pprj, \
         tc.tile_pool(name="pacc", bufs=2, space="PSUM") as pacc:

        # ---- constants ----
        ident = singles.tile([128, 128], BF16, tag="ident")
        make_identity(nc, ident[:])
        ones_f = singles.tile([128, DH + 1], F32, tag="onesf")
        nc.vector.memset(ones_f[:], 1.0)
        ones_t = singles.tile([128, DH + 1], BF16, tag="ones")
        nc.vector.tensor_copy(ones_t[:], ones_f[:])
        # preload the exp activation table (one-time ~2.7us) off the critical
        # path; in-place on ones_f[0,0] (only row 64 of ones_t is ever read,
        # and ones_t was already copied).
        nc.scalar.activation(ones_f[0:1, 0:1], ones_f[0:1, 0:1], EXP, scale=1.0)

        # ---- weight tiles, pair-sliced: only pair 0's q/k are on the
        # startup critical path, so pair 1's slices queue after the xt waves.
        # wave-0 completion is gated on the slowest queue's share, so sync
        # (which also carries wq0) gets only 2 of the 8 wave-0 halves.
        wq_sb = singles.tile([128, NPAIR, kc_n, FPP], BF16, tag="wq")
        nc.sync.dma_start(out=wq_sb[:, 0], in_=wq[:, 0])
        wk_sb = singles.tile([128, NPAIR, kc_n, FPP], BF16, tag="wk")
        wv_sb = singles.tile([128, NPAIR, kc_n, FPP], BF16, tag="wv")

        # ---- PE warm-up (HAM clock gate; see baseline notes) ----
        warm = pstp.tile([128, 1024], F32, tag="stp", name="warm")
        for _ in range(70):
            nc.tensor.matmul(warm[:, 0:128], ident[:], ident[:],
                             start=True, stop=True)

        def dummy_fillers(cnt):
            """Keep-warm PE work for filler slots with no real work left."""
            def mk():
                def f():
                    wt = pprj.tile([128, 512], F32, tag="proj", name="dum")
                    nc.tensor.matmul(wt[:, 0:128], ident[:], ident[:],
                                     start=True, stop=True)
                return f
            return [mk() for _ in range(cnt)]

        # ---- xt tile DMAs, spread across the three issue engines.
        # dma_start flow-controls on queue credits and parks its engine, so
        # the issues stay split 3 ways; gpsimd's later affine_selects and
        # scalar's k-casts are due only after its backlog drains.
        iss = [nc.sync, nc.scalar, nc.gpsimd]
        xt = {}
        idx = 0
        # half0 in two half-tile waves: the first projection only reads
        # columns 0:512 of each kc tile, so landing those 8 half-tiles
        # (1 MB) first lets the q-projection start earlier.
        for kc in range(kc_n):
            xt[0, kc] = xtp.tile([128, 1024], BF16, tag="xt",
                                 name=f"xt0_{kc}")
        # wave 0: scalar and gpsimd take 3 halves each, sync (behind wq0) 2
        wave0_eng = [nc.scalar, nc.gpsimd, nc.sync, nc.scalar,
                     nc.gpsimd, nc.sync, nc.scalar, nc.gpsimd]
        for half in range(2):
            for kc in range(kc_n):
                sl = slice(half * 512, half * 512 + 512)
                eng = wave0_eng[kc] if half == 0 else iss[idx % len(iss)]
                eng.dma_start(
                    out=xt[0, kc][:, sl], in_=xt_d[0, kc][:, sl])
                idx += 1
            if half == 0:
                nc.scalar.dma_start(out=wk_sb[:, 0], in_=wk[:, 0])
        nc.gpsimd.dma_start(out=wv_sb[:, 0], in_=wv[:, 0])
        for kc in range(kc_n):
            t = xtp.tile([128, 1024], BF16, tag="xt", name=f"xt1_{kc}")
            iss[idx % len(iss)].dma_start(out=t[:], in_=xt_d[1, kc])
            idx += 1
            xt[1, kc] = t

        # pair-1 weights + wo after all xt issues (needed mid-kernel only)
        wo_sb = singles.tile([128, NPAIR, dim], BF16, tag="wo")
        nc.sync.dma_start(out=wq_sb[:, 1], in_=wq[:, 1])
        nc.scalar.dma_start(out=wk_sb[:, 1], in_=wk[:, 1])
        nc.gpsimd.dma_start(out=wv_sb[:, 1], in_=wv[:, 1])
        nc.gpsimd.dma_start(out=wo_sb[:], in_=wo[:])

        qT = {(hp, tb): qkp.tile([128, 512], BF16, tag="qT", name=f"qT{hp}_{tb}")
              for hp in range(NPAIR) for tb in range(ntb)}
        kT = {(hp, tb): qkp.tile([128, 512], BF16, tag="kT", name=f"kT{hp}_{tb}")
              for hp in range(NPAIR) for tb in range(ntb)}
        oT = {(hp, bi): otp.tile([128, 512], BF16, tag="oT", name=f"oT{hp}_{bi}")
              for hp in range(NPAIR) for bi in range(nbi)}
        # v tiles pre-created; ones columns set once by gpsimd (SBUF-only engine)
        vtiles = {(hp, jt): vp.tile([128, 2 * DH + 2], BF16, tag="v",
                                    name=f"v{hp}_{jt}")
                  for hp in range(NPAIR) for jt in range(4 * ntb)}

        def emit_v_ones(keys, eng):
            for key in keys:
                v = vtiles[key]
                eng.memset(v[:, DH:DH + 1], 1.0)
                eng.memset(v[:, 2 * DH + 1:2 * DH + 2], 1.0)

        def xs(tb, kc):
            """xt slice for 512-token block tb, contraction chunk kc."""
            return xt[tb // 2, kc][:, (tb % 2) * 512:(tb % 2) * 512 + 512]

        def wsl(w_sb, kc, hp):
            return w_sb[:, hp, kc, :]

        def proj_chain(hp, tb, qk_pool=None):
            """Startup q/k/V chain (see baseline notes on pool choice)."""
            qk_pool = qk_pool if qk_pool is not None else pacc
            qk_tag = "proj" if qk_pool is pprj else "acc"
            for wi, (w_sb, dst) in enumerate(((wq_sb, qT[hp, tb]),
                                              (wk_sb, kT[hp, tb]))):
                ps = qk_pool.tile([128, 512], F32, tag=qk_tag, name="psqk")
                for kc in range(kc_n):
                    nc.tensor.matmul(
                        ps[:], wsl(w_sb, kc, hp), xs(tb, kc),
                        start=(kc == 0), stop=(kc == kc_n - 1))
                if wi == 1:
                    nc.scalar.copy(dst[:], ps[:])
                else:
                    nc.vector.tensor_copy(dst[:], ps[:])
            psv = pprj.tile([128, 512], F32, tag="proj", name="psv")
            for kc in range(kc_n):
                nc.tensor.matmul(
                    psv[:], wsl(wv_sb, kc, hp), xs(tb, kc),
                    start=(kc == 0), stop=(kc == kc_n - 1))
            vst = vsp.tile([128, 512], BF16, tag="vstage", name="vst")
            nc.vector.tensor_copy(vst[:], psv[:])
            for s in range(4):
                tp = pprj.tile([128, 128], BF16, tag="proj", name="tp")
                nc.tensor.transpose(tp[:], vst[:, s * 128:(s + 1) * 128], ident[:])
                v = vtiles[hp, 4 * tb + s]
                nc.vector.tensor_copy(v[:, 0:DH], tp[:, 0:DH])
                nc.vector.tensor_copy(v[:, DH + 1:2 * DH + 1], tp[:, DH:2 * DH])

        def proj_fillers(hp, tb):
            """q/k/V chain for one (hp, tb) as ~0.9us PE filler thunks."""
            st = {}

            def qk(w_sb, dstd, lo):
                def f():
                    if lo == 0:
                        st['ps'] = pprj.tile([128, 512], F32, tag="proj",
                                             name="psqk")
                    ps = st['ps']
                    for kc in range(lo, lo + kc_n // 2):
                        nc.tensor.matmul(
                            ps[:], wsl(w_sb, kc, hp), xs(tb, kc),
                            start=(kc == 0), stop=(kc == kc_n - 1))
                    if lo:
                        nc.vector.tensor_copy(dstd[:], ps[:])
                return f

            def vh(lo):
                def f():
                    if lo == 0:
                        st['psv'] = pprj.tile([128, 512], F32, tag="proj",
                                              name="psv")
                    ps = st['psv']
                    for kc in range(lo, lo + kc_n // 2):
                        nc.tensor.matmul(
                            ps[:], wsl(wv_sb, kc, hp), xs(tb, kc),
                            start=(kc == 0), stop=(kc == kc_n - 1))
                    if lo:
                        vst = vsp.tile([128, 512], BF16, tag="vstage",
                                       name="vst")
                        nc.vector.tensor_copy(vst[:], ps[:])
                        st['vst'] = vst
                return f

            def tps(s0):
                def f():
                    vst = st['vst']
                    for s in (s0, s0 + 1):
                        tp = pprj.tile([128, 128], BF16, tag="proj", name="tp")
                        nc.tensor.transpose(tp[:], vst[:, s * 128:(s + 1) * 128],
                                            ident[:])
                        v = vtiles[hp, 4 * tb + s]
                        nc.vector.tensor_copy(v[:, 0:DH], tp[:, 0:DH])
                        nc.vector.tensor_copy(v[:, DH + 1:2 * DH + 1],
                                              tp[:, DH:2 * DH])
                return f

            h = kc_n // 2
            return [qk(wq_sb, qT[hp, tb], 0), qk(wq_sb, qT[hp, tb], h),
                    qk(wk_sb, kT[hp, tb], 0), qk(wk_sb, kT[hp, tb], h),
                    vh(0), vh(h), tps(0), tps(2)]

        def outproj_fillers(bi, all_scalar=False):
            """Out-projection for 512-token block bi as 4 one-token-tile
            thunks: each accumulates BOTH head-pairs per 512-col chunk
            (4 pprj matmuls + 2 casts + 1 fused DMA).  all_scalar routes
            every cast to the scalar engine — used for the op that runs as
            filler in the FINAL attention block, so the vector FIFO stays
            clear for that block's evacuation chain (the tail's critical
            path)."""
            def mk(itl):
                def f():
                    it = 4 * bi + itl
                    ostg = ostp.tile([128, 1024], BF16, tag="outstage",
                                     name="ostg")
                    for ec in range(2):
                        ps = pprj.tile([128, 512], F32, tag="proj",
                                       name="psout")
                        for hp in range(NPAIR):
                            nc.tensor.matmul(
                                ps[:], oT[hp, bi][:, itl * 128:(itl + 1) * 128],
                                wo_sb[:, hp, ec * 512:(ec + 1) * 512],
                                start=(hp == 0), stop=(hp == NPAIR - 1))
                        if not all_scalar and (itl + ec) % 2 == 0:
                            nc.vector.tensor_copy(
                                ostg[:, ec * 512:(ec + 1) * 512], ps[:])
                        else:
                            nc.scalar.copy(
                                ostg[:, ec * 512:(ec + 1) * 512], ps[:])
                    nc.sync.dma_start(
                        out=out[it * 128:(it + 1) * 128, :], in_=ostg[:])
                return f
            return [mk(0), mk(1), mk(2), mk(3)]

        def attn_block(hp, bi, fillers=()):
            """One attention i-block (identical machinery to baseline)."""
            fillers = list(fillers)
            fi = 0
            acc = {h: pacc.tile([128, 512], F32, tag="acc", name=f"acc{h}")
                   for h in range(HPC)}
            njt = 4 * bi + 4
            for jt in range(njt):
                t = jt - 4 * bi
                w0 = 128 * t if t > 0 else 0      # first live i-column
                stp = pstp.tile([128, 1024], F32, tag="stp", name="stp")
                st3 = stp[:].rearrange("p (h i) -> p h i", h=HPC)
                for h in range(HPC):
                    nc.tensor.matmul(
                        st3[:, h, w0:512],
                        kT[hp, jt // 4][h * DH:(h + 1) * DH,
                                        (jt % 4) * 128:(jt % 4 + 1) * 128],
                        qT[hp, bi][h * DH:(h + 1) * DH, w0:512],
                        start=True, stop=True)
                pt = ptp.tile([128, 1024], BF16, tag="pt", name="pt")
                pt3 = pt[:].rearrange("p (h i) -> p h i", h=HPC)
                nc.scalar.activation(pt3[:, :, w0:512], st3[:, :, w0:512],
                                     EXP, scale=SCALE)
                if t >= 0:
                    band = pt3[:, :, 128 * t:128 * (t + 1)]
                    nc.gpsimd.affine_select(
                        out=band, in_=band,
                        compare_op=mybir.AluOpType.is_ge,
                        fill=0.0, base=0,
                        pattern=[[0, HPC], [1, 128]],
                        channel_multiplier=-1)
                for h in range(HPC):
                    nc.tensor.matmul(
                        acc[h][0:DH + 1, w0:512],
                        vtiles[hp, jt][:, h * (DH + 1):(h + 1) * (DH + 1)],
                        pt3[:, h, w0:512],
                        start=(jt == 0), stop=(jt == njt - 1))
                if fi < len(fillers):
                    fillers[fi]()
                    fi += 1
            while fi < len(fillers):
                fillers[fi]()
                fi += 1
            # keep-warm dummy bridges the evacuation latency chain
            wt = pprj.tile([128, 512], F32, tag="proj", name="dum")
            nc.tensor.matmul(wt[:, 0:128], ident[:], ident[:],
                             start=True, stop=True)
            # evacuate + normalize (O^T rows 0:64, r row 64); h1 first — its
            # path is longer (partition-shift DMA)
            for h in (1, 0):
                rrow = acc[h][DH:DH + 1, :]
                rsb = rp.tile([128, 512], BF16, tag="rsb", name="rsb")
                nc.vector.tensor_copy(rsb[DH:DH + 1, :], rrow)
                # rank-1 matmul broadcast of r across the O^T partitions.
                # h1's rb on the pstp ring (pprj was held by filler tiles,
                # serializing the tail by ~6us); h0's on pprj so the NEXT
                # block's second S tile doesn't wait for h0's reciprocal
                # (only h1's, which retires ~1.5us earlier).
                if h == 1:
                    rb = pstp.tile([128, 1024], F32, tag="stp", name="rb")
                else:
                    rb = pprj.tile([128, 512], F32, tag="proj", name="rb")
                nc.tensor.matmul(rb[0:DH, 0:512],
                                 ones_t[DH:DH + 1, 0:DH],
                                 rsb[DH:DH + 1, :],
                                 start=True, stop=True)
                rc = rp.tile([128, 512], F32, tag="rc", name="rc")
                nc.vector.reciprocal_approx_fast(rc[0:DH, :], rb[0:DH, 0:512])
                if h == 0:
                    nc.vector.tensor_mul(oT[hp, bi][0:DH, :],
                                         acc[h][0:DH, :], rc[0:DH, :])
                else:
                    st = osh.tile([128, 512], BF16, tag="ost", name="ost")
                    nc.vector.tensor_mul(st[0:DH, :], acc[h][0:DH, :],
                                         rc[0:DH, :])
                    # partition-shift via DMA; on gpsimd (light FIFO)
                    nc.gpsimd.dma_start(out=oT[hp, bi][DH:2 * DH, :],
                                        in_=st[0:DH, :])

        # schedule: block order (0,0),(0,1),(0,2),(0,3),(1,1),(1,2),(1,3),
        # (1,0) — out-proj for block bi needs BOTH head-pairs' oT, so bi=1,2
        # unlock mid-kernel and the kernel drains on the short (1,0) block
        # with only outproj(3)'s remainder + outproj(0) left.
        op = {bi: outproj_fillers(bi, all_scalar=(bi == 3))
              for bi in range(nbi)}
        pf = {(hp, tb): proj_fillers(hp, tb)
              for hp in range(NPAIR) for tb in range(ntb)
              if not (hp == 0 and tb < 2)}
        proj_chain(0, 0)
        emit_v_ones([(0, jt) for jt in range(4)], nc.vector)
        attn_block(0, 0, dummy_fillers(2))
        proj_chain(0, 1, qk_pool=pprj)
        emit_v_ones([k for k in vtiles if not (k[0] == 0 and k[1] < 4)],
                    nc.gpsimd)
        attn_block(0, 1, pf[0, 2])
        attn_block(0, 2, pf[0, 3] + pf[1, 0][:4])
        attn_block(0, 3, pf[1, 0][4:] + pf[1, 1] + pf[1, 2][:4])
        attn_block(1, 1, pf[1, 2][4:] + pf[1, 3][:4])
        attn_block(1, 2, pf[1, 3][4:] + op[1] + dummy_fillers(4))
        attn_block(1, 3, op[2] + dummy_fillers(4))
        attn_block(1, 0, op[3])
        # keep the PE clock warm across the final evacuation latency: wide
        # bridge matmuls on the pstp ring (pprj is about to be taken by
        # op[0]'s accumulators, and 128-col dummies are too sparse to keep
        # the HAM gate at 2.4 GHz — the old tail ran the final out-proj at
        # 1.2 GHz because of this)
        for _ in range(2):
            wt = pstp.tile([128, 1024], F32, tag="stp", name="bridge")
            nc.tensor.matmul(wt[:, 0:512], ident[:],
                             qT[0, 0][:, 0:512], start=True, stop=True)
            nc.tensor.matmul(wt[:, 512:1024], ident[:],
                             qT[0, 1][:, 0:512], start=True, stop=True)
        for f in op[0]:
            f()
    nc.finalize()
    return nc


def _get_nc(n, dim):
    key = (n, dim)
    if key not in _nc_cache:
        _nc_cache[key] = build_nc(n, dim)
    return _nc_cache[key]


def run_cores(x, Wq, Wkv, Wo, b, n, dim, heads):
    """Shard, run on 8 cores, return summed partial outputs (no bias)."""
    from concourse.bass_utils import run_bass_kernel_spmd
    global LAST_EXEC_NS

    import ml_dtypes
    bf16 = ml_dtypes.bfloat16

    fpc = (heads // HPB) * DH          # features per core = 256
    # tile-contiguous xt per batch: [half, kc, 128, 1024]
    xT = np.asarray(x, dtype=np.float32).transpose(0, 2, 1)   # [b, dim, n]
    xth = np.ascontiguousarray(
        xT.reshape(b, dim // 128, 128, n // 1024, 1024)
          .transpose(0, 3, 1, 2, 4)).astype(bf16)             # [b, half, kc, 128, 1024]
    Wq = np.asarray(Wq, dtype=np.float32).astype(bf16)
    Wkv = np.asarray(Wkv, dtype=np.float32).astype(bf16)
    Wo = np.asarray(Wo, dtype=np.float32).astype(bf16)
    inner = heads * DH

    def prearrange(w):
        # [dim, 256] -> [128, pair, dim//128, 128]: partition-major, with
        # each head-pair's [128, kc, 128] slice contiguous per partition
        return np.ascontiguousarray(
            w.reshape(-1, 128, 2, 128).transpose(1, 2, 0, 3))

    in_maps = []
    for c in range(NCORES):
        bb, q = divmod(c, HPB)
        sl = slice(q * fpc, (q + 1) * fpc)
        in_maps.append({
            "xt": xth[bb],
            "wq": prearrange(Wq[:, sl]),
            "wk": prearrange(Wkv[:, :inner][:, sl]),
            "wv": prearrange(Wkv[:, inner:][:, sl]),
            # [256, 1024] -> [128, 2, 1024] (pair-major rows)
            "wo": np.ascontiguousarray(
                Wo[sl, :].reshape(2, 128, dim).transpose(1, 0, 2)),
        })

    nc = _get_nc(n, dim)
    res = run_bass_kernel_spmd(nc, in_maps, core_ids=list(range(NCORES)),
                               trace=TRACE)
    LAST_EXEC_NS = res.exec_time_ns
    total = np.zeros((b, n, dim), dtype=np.float32)
    for c in range(NCORES):
        total[c // HPB] += res.results[c]["out"].astype(np.float32)
    return total


def kernel(x, Wq, Wkv, Wo, bo):
    out = run_cores(x, Wq, Wkv, Wo, B, N, DIM, HEADS)
    out += np.asarray(bo, dtype=np.float32)
    return out
